# revision 21
# baseline (speedup 1.0000x reference)
"""AdaptiveExpertRouter Trainium2 kernel (8 NeuronCores, data-parallel over tokens).

Per-core pipeline (512 tokens, feature-major activations [feat_part, tok_free]):
  - every Linear is a 3-term bf16 hi/lo split matmul (x@W = xh@Wh + xl@Wh + xh@Wl)
    accumulated in fp32 PSUM -> ~7e-6 relative error, needed so top-2 expert
    selection matches the fp32 reference on realistic score gaps
  - LayerNorm stats via ones-row matmuls (mean from bf16 hi/lo, E[x^2] from an
    fp32r Square); per-token mu/rstd broadcast back through a DRAM round-trip
    DMA (partition-replicating read) so the PE stream never stalls on stats
  - layer emission order interleaves the three scoring branches so independent
    matmul work covers every LN stats barrier and the PE stays at full clock
  - logits transposed to token-major with PE transpose, softmax + top-2 via the
    DVE max8/max_index ops, then an indirect-DMA gather of the 2 selected
    expert rows per token and an fp32 weighted combine.
"""

import os
import sys
from contextlib import ExitStack

for _p in ("/opt/trn_rl_repo", "/root/.axon_site/_ro/trn_rl_repo"):
    if os.path.isdir(_p) and _p not in sys.path:
        sys.path.append(_p)

import numpy as np
import ml_dtypes

import concourse.bass as bass
import concourse.mybir as mybir
import concourse.bacc as bacc
from concourse.tile import TileContext
from concourse.bass_utils import run_bass_kernel_spmd
from concourse.masks import make_identity

P = 128
NCORES = 8
NTOK = 4096            # B*S
CTOK = NTOK // NCORES  # 512 tokens per core
NTT = CTOK // P        # 4 token tiles per core
S_DIM = 1024
T_DIM = 4096
E = 8

f32 = mybir.dt.float32
f32r = mybir.dt.float32r
bf16 = mybir.dt.bfloat16
i32 = mybir.dt.int32
u32 = mybir.dt.uint32

bf = ml_dtypes.bfloat16

# layer name -> (in_dim, out_dim)
LAYERS = {
    "t2s":  (T_DIM, S_DIM),
    "es1":  (T_DIM, 2 * S_DIM),
    "es2":  (2 * S_DIM, P),      # logits padded 8 -> 128
    "cap1": (S_DIM, 2 * S_DIM),
    "cap2": (2 * S_DIM, S_DIM),
    "cap3": (S_DIM, P),          # logits padded
    "gap1": (2 * S_DIM, T_DIM),
    "gap2": (T_DIM, P),          # logits padded
}
LN_LAYERS = {"cap1", "cap2", "gap1"}


def _build(inv_temp: float):
    nc = bacc.Bacc("TRN2", target_bir_lowering=False, debug=False,
                   num_devices=NCORES)

    ext = {}
    ext["st_h"] = nc.dram_tensor("st_h", [S_DIM, CTOK], bf16, kind="ExternalInput")
    ext["st_l"] = nc.dram_tensor("st_l", [S_DIM, CTOK], bf16, kind="ExternalInput")
    ext["te_h"] = nc.dram_tensor("te_h", [T_DIM, CTOK], bf16, kind="ExternalInput")
    ext["te_l"] = nc.dram_tensor("te_l", [T_DIM, CTOK], bf16, kind="ExternalInput")
    for L, (ind, outd) in LAYERS.items():
        ext[f"{L}_w"] = nc.dram_tensor(f"{L}_w", [ind, 2, outd], bf16, kind="ExternalInput")
        ext[f"{L}_b"] = nc.dram_tensor(f"{L}_b", [outd, 1], f32, kind="ExternalInput")
        if L in LN_LAYERS:
            ext[f"{L}_g"] = nc.dram_tensor(f"{L}_g", [outd, 1], f32, kind="ExternalInput")
            ext[f"{L}_be"] = nc.dram_tensor(f"{L}_be", [outd, 1], f32, kind="ExternalInput")
    ext["iota"] = nc.dram_tensor("iota", [P, 1], f32, kind="ExternalInput")
    ext["eo"] = nc.dram_tensor("eo", [E * CTOK, T_DIM], f32, kind="ExternalInput")
    out_ext = nc.dram_tensor("out", [CTOK, T_DIM], f32, kind="ExternalOutput")
    # scratch DRAM for the stats partition-broadcast round trip
    bc_dram = {}
    for L in LN_LAYERS:
        bc_dram[f"{L}_mu"] = nc.dram_tensor(f"{L}_mu_d", [1, CTOK], f32)
        bc_dram[f"{L}_rstd"] = nc.dram_tensor(f"{L}_rstd_d", [1, CTOK], f32)

    with TileContext(nc) as tc, ExitStack() as top:
        const = top.enter_context(tc.tile_pool(name="const", bufs=1))
        ident = const.tile([P, P], f32, name="ident")
        make_identity(nc, ident)
        iota_sb = const.tile([P, 1], f32, name="iota_sb")
        nc.sync.dma_start(out=iota_sb[:], in_=ext["iota"][:])
        eps_t = const.tile([1, 1], f32, name="eps_t")
        nc.vector.memset(eps_t[:], 1e-5)
        ones_bf = {}
        ones_fr = {}
        for D in (S_DIM, 2 * S_DIM, T_DIM):
            tb = const.tile([P, 1], bf16, name=f"ones_bf_{D}")
            nc.vector.memset(tb[:], 1.0 / D)
            ones_bf[D] = tb
            t0 = const.tile([P, 1], f32, name=f"ones_f_{D}")
            nc.vector.memset(t0[:], 1.0 / D)
            tr = const.tile([P, 1], f32r, name=f"ones_fr_{D}")
            nc.vector.tensor_copy(out=tr[:], in_=t0[:])
            ones_fr[D] = tr

        wmix = const.tile([P, 3, 1], f32, name="wmix")
        for bi, wv in enumerate((0.4, 0.3, 0.3)):
            nc.vector.memset(wmix[:, bi, :], wv)
        biasp = top.enter_context(tc.tile_pool(name="biasp", bufs=1))
        wpool = top.enter_context(tc.tile_pool(name="wpool", bufs=6))
        lnt = top.enter_context(tc.tile_pool(name="lnt", bufs=6))
        statp = top.enter_context(tc.tile_pool(name="statp", bufs=2))
        bcp = top.enter_context(tc.tile_pool(name="bcp", bufs=2))
        lgp = top.enter_context(tc.tile_pool(name="lgp", bufs=1))
        tokp = top.enter_context(tc.tile_pool(name="tokp", bufs=40))
        # one shared-tag pool for all bf16 activation tiles: slots recycle
        # dynamically by lifetime, avoiding LIFO pool-stack constraints
        act_ctx = ExitStack()
        actp = act_ctx.enter_context(tc.tile_pool(name="actp", bufs=158))
        psT = top.enter_context(tc.tile_pool(name="psT", bufs=1, space="PSUM"))
        mm_ctx = ExitStack()
        psA = mm_ctx.enter_context(tc.tile_pool(name="psA", bufs=5, space="PSUM"))
        psS = mm_ctx.enter_context(tc.tile_pool(name="psS", bufs=2, space="PSUM"))

        def load_vec(name, outd):
            t = biasp.tile([P, outd // P, 1], f32, name=f"{name}_sb")
            nc.sync.dma_start(
                out=t[:], in_=ext[name][:].rearrange("(ot p) one -> p ot one", p=P))
            return t

        def resident_acts(pool, nm, src_h, src_l, dim):
            """fully-resident activations, one DMA per ktile; returns provider"""
            kt = dim // P
            tiles = []
            for k in range(kt):
                h = pool.tile([P, CTOK], bf16, name=f"{nm}_h{k}", tag="act")
                l = pool.tile([P, CTOK], bf16, name=f"{nm}_l{k}", tag="act")
                nc.sync.dma_start(out=h[:], in_=src_h[k * P:(k + 1) * P, :])
                nc.sync.dma_start(out=l[:], in_=src_l[k * P:(k + 1) * P, :])
                tiles.append((h[:], l[:]))

            def provider(k, og):
                return tiles[k]
            return provider

        def lazy_resident(pool, nm, src_h, src_l):
            """resident tiles DMA'd at first use (streams in under the
            consuming layer's own matmuls, reused by later ogroups)"""
            tiles = {}

            def provider(k, og):
                if k not in tiles:
                    h = pool.tile([P, CTOK], bf16, name=f"{nm}_h{k}", tag="act")
                    l = pool.tile([P, CTOK], bf16, name=f"{nm}_l{k}", tag="act")
                    nc.sync.dma_start(out=h[:], in_=src_h[k * P:(k + 1) * P, :])
                    nc.sync.dma_start(out=l[:], in_=src_l[k * P:(k + 1) * P, :])
                    tiles[k] = (h[:], l[:])
                return tiles[k]
            return provider

        def pair_provider(pairs):
            def provider(k, og):
                return pairs[k]
            return provider

        def split_linear(L, provider, epilogue):
            """3-term split matmuls for layer L; epilogue(ot, psum_ap) per
            128-row output tile. Term order reuses the wh LDWEIGHTS."""
            ind, outd = LAYERS[L]
            nkt = ind // P
            n_ot = outd // P
            for og in range(0, n_ot, 4):
                ots = list(range(og, min(og + 4, n_ot)))
                ps = {}
                for ot in ots:
                    ps[ot] = psA.tile([P, CTOK], f32, name=f"{L}_ps{ot}", tag="psA")
                for kt in range(nkt):
                    cs = slice(ots[0] * P, (ots[-1] + 1) * P)
                    w = wpool.tile([P, 2, len(ots) * P], bf16,
                                   name=f"{L}_w{og}_{kt}", tag="wblk")
                    nc.sync.dma_start(out=w[:], in_=ext[f"{L}_w"][kt * P:(kt + 1) * P, :, cs])
                    xh, xl = provider(kt, og)
                    first = kt == 0
                    last = kt == nkt - 1
                    for j, ot in enumerate(ots):
                        sl = slice(j * P, (j + 1) * P)
                        nc.tensor.matmul(ps[ot][:], lhsT=w[:, 0, sl], rhs=xh,
                                         start=first, stop=False)
                        nc.tensor.matmul(ps[ot][:], lhsT=w[:, 0, sl], rhs=xl,
                                         start=False, stop=False)
                        nc.tensor.matmul(ps[ot][:], lhsT=w[:, 1, sl], rhs=xh,
                                         start=False, stop=last)
                for ot in ots:
                    epilogue(ot, ps[ot][:])

        def plain_split_layer(L, provider, pool, gelu):
            """Linear (+bias) [+gelu], output split to bf16 hi/lo pairs."""
            _, outd = LAYERS[L]
            n_ot = outd // P
            b = load_vec(f"{L}_b", outd)
            hs, ls = [], []

            def epi(ot, psum):
                ba = b[:, ot, :]
                h = pool.tile([P, CTOK], bf16, name=f"{L}_h{ot}", tag="act")
                l = pool.tile([P, CTOK], bf16, name=f"{L}_l{ot}", tag="act")
                if gelu:
                    g32 = lnt.tile([P, CTOK], f32, name=f"{L}_g32_{ot}", tag="lnt")
                    nc.scalar.activation(g32[:], psum,
                                         mybir.ActivationFunctionType.Gelu, bias=ba)
                    nc.scalar.copy(h[:], g32[:])
                    nc.vector.tensor_sub(l[:], g32[:], h[:])
                else:
                    nc.scalar.activation(h[:], psum,
                                         mybir.ActivationFunctionType.Identity, bias=ba)
                    d1 = lnt.tile([P, CTOK], f32, name=f"{L}_d1_{ot}", tag="lnt")
                    nc.vector.tensor_sub(d1[:], psum, h[:])
                    nc.vector.tensor_scalar(l[:], d1[:], ba, None,
                                            op0=mybir.AluOpType.add)
                hs.append(h)
                ls.append(l)

            split_linear(L, provider, epi)
            return [(hs[i][:], ls[i][:]) for i in range(n_ot)]

        def ln_layer_mms(L, provider, pool):
            """Emit Linear+bias matmuls and LN stats for layer L; returns a
            finalize() that emits the normalize+GELU+split (call it after
            emitting independent PE work to cover the stats latency)."""
            _, outd = LAYERS[L]
            n_ot = outd // P
            b = load_vec(f"{L}_b", outd)
            g = load_vec(f"{L}_g", outd)
            be = load_vec(f"{L}_be", outd)
            mu_ps = psS.tile([1, CTOK], f32, name=f"{L}_mu", tag="psS")
            m2_ps = psS.tile([1, CTOK], f32, name=f"{L}_m2", tag="psS")
            yhs, yls = [], []

            def epi(ot, psum):
                ba = b[:, ot, :]
                yh = pool.tile([P, CTOK], bf16, name=f"{L}_yh{ot}", tag="act")
                yl = pool.tile([P, CTOK], bf16, name=f"{L}_yl{ot}", tag="act")
                nc.scalar.activation(yh[:], psum,
                                     mybir.ActivationFunctionType.Identity, bias=ba)
                d1 = lnt.tile([P, CTOK], f32, name=f"{L}_d1_{ot}", tag="lnt")
                nc.vector.tensor_sub(d1[:], psum, yh[:])
                nc.vector.tensor_scalar(yl[:], d1[:], ba, None,
                                        op0=mybir.AluOpType.add)
                sq = lnt.tile([P, CTOK], f32r, name=f"{L}_sq_{ot}", tag="lnt")
                nc.scalar.activation(sq[:], psum,
                                     mybir.ActivationFunctionType.Square, bias=ba)
                first = ot == 0
                last = ot == n_ot - 1
                nc.tensor.matmul(mu_ps[:], lhsT=ones_bf[outd][:], rhs=yh[:],
                                 start=first, stop=False)
                nc.tensor.matmul(mu_ps[:], lhsT=ones_bf[outd][:], rhs=yl[:],
                                 start=False, stop=last)
                nc.tensor.matmul(m2_ps[:], lhsT=ones_fr[outd][:], rhs=sq[:],
                                 start=first, stop=last)
                yhs.append(yh)
                yls.append(yl)

            split_linear(L, provider, epi)

            # stats chain (no PE instructions -> PE stream never stalls here)
            mu = statp.tile([1, CTOK], f32, name=f"{L}_mu_sb", tag="stat")
            nc.vector.tensor_copy(out=mu[:], in_=mu_ps[:])
            var = statp.tile([1, CTOK], f32, name=f"{L}_var", tag="stat")
            nc.vector.tensor_mul(var[:], mu[:], mu[:])
            nc.vector.tensor_sub(var[:], m2_ps[:], var[:])
            std = statp.tile([1, CTOK], f32, name=f"{L}_std", tag="stat")
            nc.scalar.activation(std[:], var[:],
                                 mybir.ActivationFunctionType.Sqrt, bias=eps_t[:])
            rstd = statp.tile([1, CTOK], f32, name=f"{L}_rstd", tag="stat")
            nc.vector.reciprocal(rstd[:], std[:])
            # partition-broadcast via DRAM round trip
            mu_b = bcp.tile([P, CTOK], f32, name=f"{L}_mu_b", tag="bcast")
            rstd_b = bcp.tile([P, CTOK], f32, name=f"{L}_rstd_b", tag="bcast")
            for src, key, dst in ((mu, f"{L}_mu", mu_b), (rstd, f"{L}_rstd", rstd_b)):
                dr = bc_dram[key]
                nc.sync.dma_start(out=dr[:], in_=src[:])
                nc.sync.dma_start(out=dst[:], in_=dr[:].to_broadcast([P, CTOK]))

            def finalize():
                hs, ls = [], []
                for ot in range(n_ot):
                    yh, yl = yhs[ot], yls[ot]
                    t1 = lnt.tile([P, CTOK], f32, name=f"{L}_t1_{ot}", tag="lnt")
                    nc.vector.tensor_sub(t1[:], yh[:], mu_b[:])
                    nc.vector.tensor_add(t1[:], t1[:], yl[:])
                    nc.vector.tensor_mul(t1[:], t1[:], rstd_b[:])
                    g32 = lnt.tile([P, CTOK], f32, name=f"{L}_g32_{ot}", tag="lnt")
                    nc.scalar.activation(g32[:], t1[:],
                                         mybir.ActivationFunctionType.Gelu,
                                         bias=be[:, ot, :], scale=g[:, ot, :])
                    h = pool.tile([P, CTOK], bf16, name=f"{L}_h{ot}", tag="act")
                    l = pool.tile([P, CTOK], bf16, name=f"{L}_l{ot}", tag="act")
                    nc.scalar.copy(h[:], g32[:])
                    nc.vector.tensor_sub(l[:], g32[:], h[:])
                    hs.append(h)
                    ls.append(l)
                return [(hs[i][:], ls[i][:]) for i in range(n_ot)]

            return finalize

        def logits_layer(L, provider, lgp, scale=1.0):
            b = load_vec(f"{L}_b", P)
            res = lgp.tile([P, CTOK], f32, name=f"{L}_lg")

            def epi(ot, psum):
                nc.scalar.activation(res[:], psum,
                                     mybir.ActivationFunctionType.Identity,
                                     bias=b[:, 0, :], scale=scale)

            split_linear(L, provider, epi)
            return res

        # ---------------- layer graph ----------------
        # order chosen so independent matmul streams cover every LN stats
        # barrier: cap1 -> t2s -> [cap1 fin] -> cap2 -> [cap2 fin under gap1]
        # -> gap1 -> cap3 -> es1 (covers gap1 finalize) -> gap2 -> es2
        st = resident_acts(actp, "st", ext["st_h"], ext["st_l"], S_DIM)

        cap1_fin = ln_layer_mms("cap1", st, actp)
        cap1 = cap1_fin()  # no PE ops; DVE work overlaps t2s matmuls below

        te1 = lazy_resident(actp, "te1", ext["te_h"], ext["te_l"])
        t2s = plain_split_layer("t2s", te1, actp, gelu=False)

        cap2_fin = ln_layer_mms("cap2", pair_provider(cap1), actp)
        cap2 = cap2_fin()  # DVE work overlaps gap1 matmuls below

        def gap_in(k, og):
            if k < S_DIM // P:
                return st(k, og)
            return t2s[k - S_DIM // P]

        gap1_fin = ln_layer_mms("gap1", gap_in, actp)

        lt3 = [tokp.tile([P, 3, E], f32, name=f"lt3_{tt}", tag="lt3")
               for tt in range(NTT)]

        def branch_transpose(lg, br):
            for tt in range(NTT):
                pst = psT.tile([P, P], f32, name=f"tr{br}_{tt}", tag="psT")
                nc.tensor.transpose(out=pst[:], in_=lg[:, tt * P:(tt + 1) * P],
                                    identity=ident[:])
                nc.vector.tensor_copy(out=lt3[tt][:, br, :], in_=pst[:, 0:E])

        lg_cap = logits_layer("cap3", pair_provider(cap2), lgp, scale=inv_temp)
        branch_transpose(lg_cap, 0)

        te2 = lazy_resident(actp, "te2", ext["te_h"], ext["te_l"])
        for _k in range(6):  # prefetch first ktiles so es1 starts DMA-warm
            te2(_k, 0)
        gap1 = gap1_fin()

        es1 = plain_split_layer("es1", te2, actp, gelu=True)

        lg_gap = logits_layer("gap2", pair_provider(gap1), lgp)
        branch_transpose(lg_gap, 1)
        lg_es = logits_layer("es2", pair_provider(es1), lgp)
        branch_transpose(lg_es, 2)

        # ---------------- token-major epilogue ----------------
        act_ctx.close()  # release activation SBUF for gather/out buffers
        mm_ctx.close()  # release matmul-phase PSUM banks
        gp = top.enter_context(tc.tile_pool(name="gp", bufs=4))
        outp = top.enter_context(tc.tile_pool(name="outp", bufs=4))

        for tt in range(NTT):
            # batched softmax over the 3 branches: lt3[tt] is [P, 3, 8]
            lv = lt3[tt][:]
            m3 = tokp.tile([P, 3, 1], f32, name=f"m3_{tt}", tag="tok3")
            nc.vector.reduce_max(out=m3[:], in_=lv, axis=mybir.AxisListType.X)
            d3 = tokp.tile([P, 3, E], f32, name=f"d3_{tt}", tag="lt3")
            nc.vector.tensor_sub(d3[:], lv, m3[:].to_broadcast([P, 3, E]))
            e3 = tokp.tile([P, 3, E], f32, name=f"e3_{tt}", tag="lt3")
            nc.scalar.activation(e3[:], d3[:], mybir.ActivationFunctionType.Exp)
            den3 = tokp.tile([P, 3, 1], f32, name=f"den3_{tt}", tag="tok3")
            nc.vector.reduce_sum(out=den3[:], in_=e3[:], axis=mybir.AxisListType.X)
            rd3 = tokp.tile([P, 3, 1], f32, name=f"rd3_{tt}", tag="tok3")
            nc.vector.reciprocal(rd3[:], den3[:])
            wd3 = tokp.tile([P, 3, 1], f32, name=f"wd3_{tt}", tag="tok3")
            nc.vector.tensor_mul(wd3[:], rd3[:], wmix[:])
            p3 = tokp.tile([P, 3, E], f32, name=f"p3_{tt}", tag="lt3")
            nc.vector.tensor_mul(p3[:], e3[:], wd3[:].to_broadcast([P, 3, E]))
            comb = tokp.tile([P, E], f32, name=f"comb_{tt}", tag="tok")
            nc.vector.tensor_add(comb[:], p3[:, 0, :], p3[:, 1, :])
            nc.vector.tensor_add(comb[:], comb[:], p3[:, 2, :])

            vals = tokp.tile([P, 8], f32, name=f"vals_{tt}", tag="tok")
            nc.vector.max(out=vals[:], in_=comb[:])
            idx = tokp.tile([P, 8], u32, name=f"idx_{tt}", tag="tok")
            nc.vector.max_index(out=idx[:], in_max=vals[:], in_values=comb[:])

            den2 = tokp.tile([P, 1], f32, name=f"den2_{tt}", tag="tok1")
            nc.vector.tensor_add(den2[:], vals[:, 0:1], vals[:, 1:2])
            nc.vector.tensor_scalar(den2[:], den2[:], 1e-8, None,
                                    op0=mybir.AluOpType.add)
            rden = tokp.tile([P, 1], f32, name=f"rden_{tt}", tag="tok1")
            nc.vector.reciprocal(rden[:], den2[:])
            w1 = tokp.tile([P, 1], f32, name=f"w1_{tt}", tag="tok1")
            w2 = tokp.tile([P, 1], f32, name=f"w2_{tt}", tag="tok1")
            nc.vector.tensor_mul(w1[:], vals[:, 0:1], rden[:])
            nc.vector.tensor_mul(w2[:], vals[:, 1:2], rden[:])

            idf = tokp.tile([P, 2], f32, name=f"idf_{tt}", tag="tok")
            nc.vector.tensor_copy(out=idf[:], in_=idx[:, 0:2])
            off_f = tokp.tile([P, 2], f32, name=f"offf_{tt}", tag="tok")
            nc.vector.tensor_scalar(off_f[:], idf[:], float(CTOK), iota_sb[:],
                                    op0=mybir.AluOpType.mult,
                                    op1=mybir.AluOpType.add)
            nc.vector.tensor_scalar(off_f[:], off_f[:], float(tt * P), None,
                                    op0=mybir.AluOpType.add)
            off_i = tokp.tile([P, 2], i32, name=f"offi_{tt}", tag="tok")
            nc.vector.tensor_copy(out=off_i[:], in_=off_f[:])
            gs = []
            for j in (0, 1):
                gt = gp.tile([P, T_DIM], f32, name=f"g{j}_{tt}", tag="gather")
                nc.gpsimd.indirect_dma_start(
                    out=gt[:], out_offset=None, in_=ext["eo"][:],
                    in_offset=bass.IndirectOffsetOnAxis(ap=off_i[:, j:j + 1], axis=0))
                gs.append(gt)

            o1 = outp.tile([P, T_DIM], f32, name=f"o1_{tt}", tag="outb")
            nc.scalar.activation(o1[:], gs[0][:],
                                 mybir.ActivationFunctionType.Copy,
                                 scale=w1[:])
            o2 = outp.tile([P, T_DIM], f32, name=f"o2_{tt}", tag="outb")
            nc.scalar.activation(o2[:], gs[1][:],
                                 mybir.ActivationFunctionType.Copy,
                                 scale=w2[:])
            nc.vector.tensor_add(o1[:], o1[:], o2[:])
            nc.sync.dma_start(out=out_ext[tt * P:(tt + 1) * P, :], in_=o1[:])

    nc.compile()
    return nc


def _split(a):
    h = a.astype(bf)
    l = (a - h.astype(np.float32)).astype(bf)
    return h, l


def _prep_inputs(student_hidden, expert_outputs, params):
    """host-side prep: shard + transpose + hi/lo split + weight layout"""
    p = {k: np.asarray(v, dtype=np.float32) for k, v in params.items()}
    sh = np.ascontiguousarray(np.asarray(student_hidden, np.float32).reshape(NTOK, S_DIM))
    eo = np.asarray(expert_outputs, np.float32).reshape(E, NTOK, T_DIM)

    wmap = {
        "t2s": ("t2s_w", "t2s_b"), "es1": ("es_w1", "es_b1"), "es2": ("es_w2", "es_b2"),
        "cap1": ("cap_w1", "cap_b1"), "cap2": ("cap_w2", "cap_b2"), "cap3": ("cap_w3", "cap_b3"),
        "gap1": ("gap_w1", "gap_b1"), "gap2": ("gap_w2", "gap_b2"),
    }
    shared = {}
    for L, (wk, bk) in wmap.items():
        ind, outd = LAYERS[L]
        w = p[wk]  # [out, in] torch convention
        wT = np.ascontiguousarray(w.T)
        b = p[bk]
        if wT.shape[1] < outd:  # pad logits layers to 128 outputs
            wT = np.concatenate(
                [wT, np.zeros((ind, outd - wT.shape[1]), np.float32)], axis=1)
            b = np.concatenate([b, np.zeros(outd - b.shape[0], np.float32)])
        wh, wl = _split(wT)
        shared[f"{L}_w"] = np.ascontiguousarray(np.stack([wh, wl], axis=1))
        shared[f"{L}_b"] = np.ascontiguousarray(b.reshape(outd, 1))
        if L in LN_LAYERS:
            shared[f"{L}_g"] = np.ascontiguousarray(p[wk.replace("_w", "_g")].reshape(outd, 1))
            ben = {"cap1": "cap_be1", "cap2": "cap_be2", "gap1": "gap_be1"}[L]
            shared[f"{L}_be"] = np.ascontiguousarray(p[ben].reshape(outd, 1))
    shared["iota"] = np.arange(P, dtype=np.float32).reshape(P, 1)
    inv_temp = float(1.0 / p["temp"].reshape(-1)[0])
    shared["cap3_b"] = np.ascontiguousarray(shared["cap3_b"] * inv_temp)

    in_maps = []
    for c in range(NCORES):
        cs = slice(c * CTOK, (c + 1) * CTOK)
        m = dict(shared)
        m["st_h"], m["st_l"] = _split(np.ascontiguousarray(sh[cs].T))
        m["te_h"], m["te_l"] = _split(np.ascontiguousarray(eo[0, cs].T))
        m["eo"] = np.ascontiguousarray(eo[:, cs, :]).reshape(E * CTOK, T_DIM)
        in_maps.append(m)
    return in_maps, inv_temp


_CACHE = {}


def kernel(student_hidden, expert_outputs, params):
    in_maps, inv_temp = _prep_inputs(student_hidden, expert_outputs, params)
    key = ("nc", inv_temp)
    if key not in _CACHE:
        _CACHE[key] = _build(inv_temp)
    nc = _CACHE[key]
    trace = bool(int(os.environ.get("KERNEL_TRACE", "0")))
    tmpdir = os.environ.get("KERNEL_TRACE_DIR") or None
    res = run_bass_kernel_spmd(nc, in_maps, core_ids=list(range(NCORES)),
                               trace=trace, tmpdir=tmpdir)
    if trace:
        kernel.last_exec_time_ns = res.exec_time_ns
        kernel.last_results = res
    out = np.concatenate([res.results[c]["out"] for c in range(NCORES)], axis=0)
    return out.reshape(2, NTOK // 2, T_DIM)


kernel.last_exec_time_ns = None
kernel.last_results = None


# revision 22
# speedup vs baseline: 1.1695x; 1.1695x over previous
"""AdaptiveExpertRouter Trainium2 kernel (8 NeuronCores, data-parallel over tokens).

Per-core pipeline (512 tokens, feature-major activations [feat_part, tok_free]):
  - every Linear is a 3-term bf16 hi/lo split matmul (x@W = xh@Wh + xl@Wh + xh@Wl)
    accumulated in fp32 PSUM -> ~7e-6 relative error, needed so top-2 expert
    selection matches the fp32 reference on realistic score gaps
  - LayerNorm stats via ones-row matmuls (mean from bf16 hi/lo, E[x^2] from an
    fp32r Square); per-token mu/rstd broadcast back through a DRAM round-trip
    DMA (partition-replicating read) so the PE stream never stalls on stats
  - layer emission order interleaves the three scoring branches so independent
    matmul work covers every LN stats barrier and the PE stays at full clock
  - logits transposed to token-major with PE transpose, softmax + top-2 via the
    DVE max8/max_index ops, then an indirect-DMA gather of the 2 selected
    expert rows per token and an fp32 weighted combine.
"""

import os
import sys
from contextlib import ExitStack

for _p in ("/opt/trn_rl_repo", "/root/.axon_site/_ro/trn_rl_repo"):
    if os.path.isdir(_p) and _p not in sys.path:
        sys.path.append(_p)

import numpy as np
import ml_dtypes

import concourse.bass as bass
import concourse.mybir as mybir
import concourse.bacc as bacc
from concourse.tile import TileContext
from concourse.bass_utils import run_bass_kernel_spmd
from concourse.masks import make_identity

P = 128
NCORES = 8
NTOK = 4096            # B*S
CTOK = NTOK // NCORES  # 512 tokens per core
NTT = CTOK // P        # 4 token tiles per core
S_DIM = 1024
T_DIM = 4096
E = 8

f32 = mybir.dt.float32
f32r = mybir.dt.float32r
bf16 = mybir.dt.bfloat16
i32 = mybir.dt.int32
u32 = mybir.dt.uint32

bf = ml_dtypes.bfloat16

# layer name -> (in_dim, out_dim)
LAYERS = {
    "t2s":  (T_DIM, S_DIM),
    "es1":  (T_DIM, 2 * S_DIM),
    "es2":  (2 * S_DIM, P),      # logits padded 8 -> 128
    "cap1": (S_DIM, 2 * S_DIM),
    "cap2": (2 * S_DIM, S_DIM),
    "cap3": (S_DIM, P),          # logits padded
    "gap1": (2 * S_DIM, T_DIM),
    "gap2": (T_DIM, P),          # logits padded
}
LN_LAYERS = {"cap1", "cap2", "gap1"}


def _build(inv_temp: float):
    nc = bacc.Bacc("TRN2", target_bir_lowering=False, debug=False,
                   num_devices=NCORES)

    ext = {}
    ext["st_h"] = nc.dram_tensor("st_h", [S_DIM, CTOK], bf16, kind="ExternalInput")
    ext["st_l"] = nc.dram_tensor("st_l", [S_DIM, CTOK], bf16, kind="ExternalInput")
    ext["te_h"] = nc.dram_tensor("te_h", [T_DIM, CTOK], bf16, kind="ExternalInput")
    ext["te_l"] = nc.dram_tensor("te_l", [T_DIM, CTOK], bf16, kind="ExternalInput")
    for L, (ind, outd) in LAYERS.items():
        ext[f"{L}_w"] = nc.dram_tensor(f"{L}_w", [ind, 2, outd], bf16, kind="ExternalInput")
        ext[f"{L}_b"] = nc.dram_tensor(f"{L}_b", [outd, 1], f32, kind="ExternalInput")
        if L in LN_LAYERS:
            ext[f"{L}_g"] = nc.dram_tensor(f"{L}_g", [outd, 1], f32, kind="ExternalInput")
            ext[f"{L}_be"] = nc.dram_tensor(f"{L}_be", [outd, 1], f32, kind="ExternalInput")
    ext["iota"] = nc.dram_tensor("iota", [P, 1], f32, kind="ExternalInput")
    ext["eo"] = nc.dram_tensor("eo", [E * CTOK, T_DIM], f32, kind="ExternalInput")
    out_ext = nc.dram_tensor("out", [CTOK, T_DIM], f32, kind="ExternalOutput")
    # scratch DRAM for the stats partition-broadcast round trip
    bc_dram = {}
    for L in LN_LAYERS:
        bc_dram[f"{L}_mu"] = nc.dram_tensor(f"{L}_mu_d", [1, CTOK], f32)
        bc_dram[f"{L}_rstd"] = nc.dram_tensor(f"{L}_rstd_d", [1, CTOK], f32)

    with TileContext(nc) as tc, ExitStack() as top:
        const = top.enter_context(tc.tile_pool(name="const", bufs=1))
        ident = const.tile([P, P], f32, name="ident")
        make_identity(nc, ident)
        iota_sb = const.tile([P, 1], f32, name="iota_sb")
        nc.sync.dma_start(out=iota_sb[:], in_=ext["iota"][:])
        eps_t = const.tile([1, 1], f32, name="eps_t")
        nc.vector.memset(eps_t[:], 1e-5)
        ones_bf = {}
        ones_fr = {}
        for D in (S_DIM, 2 * S_DIM, T_DIM):
            tb = const.tile([P, 1], bf16, name=f"ones_bf_{D}")
            nc.vector.memset(tb[:], 1.0 / D)
            ones_bf[D] = tb
            t0 = const.tile([P, 1], f32, name=f"ones_f_{D}")
            nc.vector.memset(t0[:], 1.0 / D)
            tr = const.tile([P, 1], f32r, name=f"ones_fr_{D}")
            nc.vector.tensor_copy(out=tr[:], in_=t0[:])
            ones_fr[D] = tr

        wmix = const.tile([P, 3, 1], f32, name="wmix")
        for bi, wv in enumerate((0.4, 0.3, 0.3)):
            nc.vector.memset(wmix[:, bi, :], wv)
        biasp = top.enter_context(tc.tile_pool(name="biasp", bufs=1))
        wpool = top.enter_context(tc.tile_pool(name="wpool", bufs=6))
        lnt = top.enter_context(tc.tile_pool(name="lnt", bufs=6))
        statp = top.enter_context(tc.tile_pool(name="statp", bufs=2))
        bcp = top.enter_context(tc.tile_pool(name="bcp", bufs=2))
        lgp = top.enter_context(tc.tile_pool(name="lgp", bufs=1))
        tokp = top.enter_context(tc.tile_pool(name="tokp", bufs=40))
        # one shared-tag pool for all bf16 activation tiles: slots recycle
        # dynamically by lifetime, avoiding LIFO pool-stack constraints
        act_ctx = ExitStack()
        actp = act_ctx.enter_context(tc.tile_pool(name="actp", bufs=158))
        psT = top.enter_context(tc.tile_pool(name="psT", bufs=1, space="PSUM"))
        mm_ctx = ExitStack()
        psA = mm_ctx.enter_context(tc.tile_pool(name="psA", bufs=5, space="PSUM"))
        psS = mm_ctx.enter_context(tc.tile_pool(name="psS", bufs=2, space="PSUM"))

        def load_vec(name, outd):
            t = biasp.tile([P, outd // P, 1], f32, name=f"{name}_sb")
            nc.sync.dma_start(
                out=t[:], in_=ext[name][:].rearrange("(ot p) one -> p ot one", p=P))
            return t

        def resident_acts(pool, nm, src_h, src_l, dim):
            """fully-resident activations, one DMA per ktile; returns provider"""
            kt = dim // P
            tiles = []
            for k in range(kt):
                h = pool.tile([P, CTOK], bf16, name=f"{nm}_h{k}", tag="act")
                l = pool.tile([P, CTOK], bf16, name=f"{nm}_l{k}", tag="act")
                nc.sync.dma_start(out=h[:], in_=src_h[k * P:(k + 1) * P, :])
                nc.sync.dma_start(out=l[:], in_=src_l[k * P:(k + 1) * P, :])
                tiles.append((h[:], l[:]))

            def provider(k, og):
                return tiles[k]
            return provider

        def lazy_resident(pool, nm, src_h, src_l):
            """resident tiles DMA'd at first use (streams in under the
            consuming layer's own matmuls, reused by later ogroups)"""
            tiles = {}

            def provider(k, og):
                if k not in tiles:
                    h = pool.tile([P, CTOK], bf16, name=f"{nm}_h{k}", tag="act")
                    l = pool.tile([P, CTOK], bf16, name=f"{nm}_l{k}", tag="act")
                    nc.sync.dma_start(out=h[:], in_=src_h[k * P:(k + 1) * P, :])
                    nc.sync.dma_start(out=l[:], in_=src_l[k * P:(k + 1) * P, :])
                    tiles[k] = (h[:], l[:])
                return tiles[k]
            return provider

        def pair_provider(pairs):
            def provider(k, og):
                return pairs[k]
            return provider

        def split_linear(L, provider, epilogue):
            """3-term split matmuls for layer L; epilogue(ot, psum_ap) per
            128-row output tile. Term order reuses the wh LDWEIGHTS."""
            ind, outd = LAYERS[L]
            nkt = ind // P
            n_ot = outd // P
            for og in range(0, n_ot, 4):
                ots = list(range(og, min(og + 4, n_ot)))
                ps = {}
                for ot in ots:
                    ps[ot] = psA.tile([P, CTOK], f32, name=f"{L}_ps{ot}", tag="psA")
                for kt in range(nkt):
                    cs = slice(ots[0] * P, (ots[-1] + 1) * P)
                    w = wpool.tile([P, 2, len(ots) * P], bf16,
                                   name=f"{L}_w{og}_{kt}", tag="wblk")
                    nc.sync.dma_start(out=w[:], in_=ext[f"{L}_w"][kt * P:(kt + 1) * P, :, cs])
                    xh, xl = provider(kt, og)
                    first = kt == 0
                    last = kt == nkt - 1
                    for j, ot in enumerate(ots):
                        sl = slice(j * P, (j + 1) * P)
                        nc.tensor.matmul(ps[ot][:], lhsT=w[:, 0, sl], rhs=xh,
                                         start=first, stop=False)
                        nc.tensor.matmul(ps[ot][:], lhsT=w[:, 0, sl], rhs=xl,
                                         start=False, stop=False)
                        nc.tensor.matmul(ps[ot][:], lhsT=w[:, 1, sl], rhs=xh,
                                         start=False, stop=last)
                for ot in ots:
                    epilogue(ot, ps[ot][:])

        def plain_split_layer(L, provider, pool, gelu):
            """Linear (+bias) [+gelu], output split to bf16 hi/lo pairs."""
            _, outd = LAYERS[L]
            n_ot = outd // P
            b = load_vec(f"{L}_b", outd)
            hs, ls = [], []

            def epi(ot, psum):
                ba = b[:, ot, :]
                h = pool.tile([P, CTOK], bf16, name=f"{L}_h{ot}", tag="act")
                l = pool.tile([P, CTOK], bf16, name=f"{L}_l{ot}", tag="act")
                if gelu:
                    g32 = lnt.tile([P, CTOK], f32, name=f"{L}_g32_{ot}", tag="lnt")
                    nc.scalar.activation(g32[:], psum,
                                         mybir.ActivationFunctionType.Gelu, bias=ba)
                    nc.scalar.copy(h[:], g32[:])
                    nc.vector.tensor_sub(l[:], g32[:], h[:])
                else:
                    nc.scalar.activation(h[:], psum,
                                         mybir.ActivationFunctionType.Identity, bias=ba)
                    nc.vector.scalar_tensor_tensor(
                        l[:], psum, ba, h[:],
                        op0=mybir.AluOpType.add, op1=mybir.AluOpType.subtract)
                hs.append(h)
                ls.append(l)

            split_linear(L, provider, epi)
            return [(hs[i][:], ls[i][:]) for i in range(n_ot)]

        def ln_layer_mms(L, provider, pool):
            """Emit Linear+bias matmuls and LN stats for layer L; returns a
            finalize() that emits the normalize+GELU+split (call it after
            emitting independent PE work to cover the stats latency)."""
            _, outd = LAYERS[L]
            n_ot = outd // P
            b = load_vec(f"{L}_b", outd)
            g = load_vec(f"{L}_g", outd)
            be = load_vec(f"{L}_be", outd)
            mu_ps = psS.tile([1, CTOK], f32, name=f"{L}_mu", tag="psS")
            m2_ps = psS.tile([1, CTOK], f32, name=f"{L}_m2", tag="psS")
            yhs, yls = [], []

            def epi(ot, psum):
                ba = b[:, ot, :]
                yh = pool.tile([P, CTOK], bf16, name=f"{L}_yh{ot}", tag="act")
                yl = pool.tile([P, CTOK], bf16, name=f"{L}_yl{ot}", tag="act")
                nc.scalar.activation(yh[:], psum,
                                     mybir.ActivationFunctionType.Identity, bias=ba)
                nc.vector.scalar_tensor_tensor(
                    yl[:], psum, ba, yh[:],
                    op0=mybir.AluOpType.add, op1=mybir.AluOpType.subtract)
                sq = lnt.tile([P, CTOK], f32r, name=f"{L}_sq_{ot}", tag="lnt")
                nc.scalar.activation(sq[:], psum,
                                     mybir.ActivationFunctionType.Square, bias=ba)
                first = ot == 0
                last = ot == n_ot - 1
                nc.tensor.matmul(mu_ps[:], lhsT=ones_bf[outd][:], rhs=yh[:],
                                 start=first, stop=False)
                nc.tensor.matmul(mu_ps[:], lhsT=ones_bf[outd][:], rhs=yl[:],
                                 start=False, stop=last)
                nc.tensor.matmul(m2_ps[:], lhsT=ones_fr[outd][:], rhs=sq[:],
                                 start=first, stop=last)
                yhs.append(yh)
                yls.append(yl)

            split_linear(L, provider, epi)

            # stats chain (no PE instructions -> PE stream never stalls here)
            mu = statp.tile([1, CTOK], f32, name=f"{L}_mu_sb", tag="stat")
            nc.vector.tensor_copy(out=mu[:], in_=mu_ps[:])
            var = statp.tile([1, CTOK], f32, name=f"{L}_var", tag="stat")
            nc.vector.tensor_mul(var[:], mu[:], mu[:])
            nc.vector.tensor_sub(var[:], m2_ps[:], var[:])
            std = statp.tile([1, CTOK], f32, name=f"{L}_std", tag="stat")
            nc.scalar.activation(std[:], var[:],
                                 mybir.ActivationFunctionType.Sqrt, bias=eps_t[:])
            rstd = statp.tile([1, CTOK], f32, name=f"{L}_rstd", tag="stat")
            nc.vector.reciprocal(rstd[:], std[:])
            # partition-broadcast via DRAM round trip
            mu_b = bcp.tile([P, CTOK], f32, name=f"{L}_mu_b", tag="bcast")
            rstd_b = bcp.tile([P, CTOK], f32, name=f"{L}_rstd_b", tag="bcast")
            for src, key, dst in ((mu, f"{L}_mu", mu_b), (rstd, f"{L}_rstd", rstd_b)):
                dr = bc_dram[key]
                nc.sync.dma_start(out=dr[:], in_=src[:])
                nc.sync.dma_start(out=dst[:], in_=dr[:].to_broadcast([P, CTOK]))

            def finalize():
                hs, ls = [], []
                for ot in range(n_ot):
                    yh, yl = yhs[ot], yls[ot]
                    t1 = lnt.tile([P, CTOK], f32, name=f"{L}_t1_{ot}", tag="lnt")
                    nc.vector.tensor_sub(t1[:], yh[:], mu_b[:])
                    nc.vector.tensor_add(t1[:], t1[:], yl[:])
                    nc.vector.tensor_mul(t1[:], t1[:], rstd_b[:])
                    g32 = lnt.tile([P, CTOK], f32, name=f"{L}_g32_{ot}", tag="lnt")
                    nc.scalar.activation(g32[:], t1[:],
                                         mybir.ActivationFunctionType.Gelu,
                                         bias=be[:, ot, :], scale=g[:, ot, :])
                    h = pool.tile([P, CTOK], bf16, name=f"{L}_h{ot}", tag="act")
                    l = pool.tile([P, CTOK], bf16, name=f"{L}_l{ot}", tag="act")
                    nc.scalar.copy(h[:], g32[:])
                    nc.vector.tensor_sub(l[:], g32[:], h[:])
                    hs.append(h)
                    ls.append(l)
                return [(hs[i][:], ls[i][:]) for i in range(n_ot)]

            return finalize

        def logits_layer(L, provider, lgp, scale=1.0):
            b = load_vec(f"{L}_b", P)
            res = lgp.tile([P, CTOK], f32, name=f"{L}_lg")

            def epi(ot, psum):
                nc.scalar.activation(res[:], psum,
                                     mybir.ActivationFunctionType.Identity,
                                     bias=b[:, 0, :], scale=scale)

            split_linear(L, provider, epi)
            return res

        # ---------------- layer graph ----------------
        # order chosen so independent matmul streams cover every LN stats
        # barrier: cap1 -> t2s -> [cap1 fin] -> cap2 -> [cap2 fin under gap1]
        # -> gap1 -> cap3 -> es1 (covers gap1 finalize) -> gap2 -> es2
        st = resident_acts(actp, "st", ext["st_h"], ext["st_l"], S_DIM)

        cap1_fin = ln_layer_mms("cap1", st, actp)
        cap1 = cap1_fin()  # no PE ops; DVE work overlaps t2s matmuls below

        te1 = resident_acts(actp, "te1", ext["te_h"], ext["te_l"], T_DIM)
        t2s = plain_split_layer("t2s", te1, actp, gelu=False)

        cap2_fin = ln_layer_mms("cap2", pair_provider(cap1), actp)
        cap2 = cap2_fin()  # DVE work overlaps gap1 matmuls below

        def gap_in(k, og):
            if k < S_DIM // P:
                return st(k, og)
            return t2s[k - S_DIM // P]

        gap1_fin = ln_layer_mms("gap1", gap_in, actp)

        lt3 = [tokp.tile([P, 3, E], f32, name=f"lt3_{tt}", tag="lt3")
               for tt in range(NTT)]

        def branch_transpose(lg, br):
            for tt in range(NTT):
                pst = psT.tile([P, P], f32, name=f"tr{br}_{tt}", tag="psT")
                nc.tensor.transpose(out=pst[:], in_=lg[:, tt * P:(tt + 1) * P],
                                    identity=ident[:])
                nc.vector.tensor_copy(out=lt3[tt][:, br, :], in_=pst[:, 0:E])

        lg_cap = logits_layer("cap3", pair_provider(cap2), lgp, scale=inv_temp)
        branch_transpose(lg_cap, 0)

        gap1 = gap1_fin()

        te2 = lazy_resident(actp, "te2", ext["te_h"], ext["te_l"])
        es1 = plain_split_layer("es1", te2, actp, gelu=True)

        lg_es = logits_layer("es2", pair_provider(es1), lgp)
        branch_transpose(lg_es, 2)
        lg_gap = logits_layer("gap2", pair_provider(gap1), lgp)
        branch_transpose(lg_gap, 1)

        # ---------------- token-major epilogue ----------------
        act_ctx.close()  # release activation SBUF for gather/out buffers
        mm_ctx.close()  # release matmul-phase PSUM banks
        gp = top.enter_context(tc.tile_pool(name="gp", bufs=4))
        outp = top.enter_context(tc.tile_pool(name="outp", bufs=4))

        for tt in range(NTT):
            # batched softmax over the 3 branches: lt3[tt] is [P, 3, 8]
            lv = lt3[tt][:]
            m3 = tokp.tile([P, 3, 1], f32, name=f"m3_{tt}", tag="tok3")
            nc.vector.reduce_max(out=m3[:], in_=lv, axis=mybir.AxisListType.X)
            d3 = tokp.tile([P, 3, E], f32, name=f"d3_{tt}", tag="lt3")
            nc.vector.tensor_sub(d3[:], lv, m3[:].to_broadcast([P, 3, E]))
            e3 = tokp.tile([P, 3, E], f32, name=f"e3_{tt}", tag="lt3")
            nc.scalar.activation(e3[:], d3[:], mybir.ActivationFunctionType.Exp)
            den3 = tokp.tile([P, 3, 1], f32, name=f"den3_{tt}", tag="tok3")
            nc.vector.reduce_sum(out=den3[:], in_=e3[:], axis=mybir.AxisListType.X)
            rd3 = tokp.tile([P, 3, 1], f32, name=f"rd3_{tt}", tag="tok3")
            nc.vector.reciprocal(rd3[:], den3[:])
            wd3 = tokp.tile([P, 3, 1], f32, name=f"wd3_{tt}", tag="tok3")
            nc.vector.tensor_mul(wd3[:], rd3[:], wmix[:])
            p3 = tokp.tile([P, 3, E], f32, name=f"p3_{tt}", tag="lt3")
            nc.vector.tensor_mul(p3[:], e3[:], wd3[:].to_broadcast([P, 3, E]))
            comb = tokp.tile([P, E], f32, name=f"comb_{tt}", tag="tok")
            nc.vector.tensor_add(comb[:], p3[:, 0, :], p3[:, 1, :])
            nc.vector.tensor_add(comb[:], comb[:], p3[:, 2, :])

            vals = tokp.tile([P, 8], f32, name=f"vals_{tt}", tag="tok")
            nc.vector.max(out=vals[:], in_=comb[:])
            idx = tokp.tile([P, 8], u32, name=f"idx_{tt}", tag="tok")
            nc.vector.max_index(out=idx[:], in_max=vals[:], in_values=comb[:])

            den2 = tokp.tile([P, 1], f32, name=f"den2_{tt}", tag="tok1")
            nc.vector.tensor_add(den2[:], vals[:, 0:1], vals[:, 1:2])
            nc.vector.tensor_scalar(den2[:], den2[:], 1e-8, None,
                                    op0=mybir.AluOpType.add)
            rden = tokp.tile([P, 1], f32, name=f"rden_{tt}", tag="tok1")
            nc.vector.reciprocal(rden[:], den2[:])
            w1 = tokp.tile([P, 1], f32, name=f"w1_{tt}", tag="tok1")
            w2 = tokp.tile([P, 1], f32, name=f"w2_{tt}", tag="tok1")
            nc.vector.tensor_mul(w1[:], vals[:, 0:1], rden[:])
            nc.vector.tensor_mul(w2[:], vals[:, 1:2], rden[:])

            idf = tokp.tile([P, 2], f32, name=f"idf_{tt}", tag="tok")
            nc.vector.tensor_copy(out=idf[:], in_=idx[:, 0:2])
            off_f = tokp.tile([P, 2], f32, name=f"offf_{tt}", tag="tok")
            nc.vector.tensor_scalar(off_f[:], idf[:], float(CTOK), iota_sb[:],
                                    op0=mybir.AluOpType.mult,
                                    op1=mybir.AluOpType.add)
            nc.vector.tensor_scalar(off_f[:], off_f[:], float(tt * P), None,
                                    op0=mybir.AluOpType.add)
            off_i = tokp.tile([P, 2], i32, name=f"offi_{tt}", tag="tok")
            nc.vector.tensor_copy(out=off_i[:], in_=off_f[:])
            gs = []
            for j in (0, 1):
                gt = gp.tile([P, T_DIM], f32, name=f"g{j}_{tt}", tag="gather")
                nc.gpsimd.indirect_dma_start(
                    out=gt[:], out_offset=None, in_=ext["eo"][:],
                    in_offset=bass.IndirectOffsetOnAxis(ap=off_i[:, j:j + 1], axis=0))
                gs.append(gt)

            o1 = outp.tile([P, T_DIM], f32, name=f"o1_{tt}", tag="outb")
            nc.scalar.activation(o1[:], gs[0][:],
                                 mybir.ActivationFunctionType.Copy,
                                 scale=w1[:])
            o2 = outp.tile([P, T_DIM], f32, name=f"o2_{tt}", tag="outb")
            nc.vector.scalar_tensor_tensor(
                o2[:], gs[1][:], w2[:], o1[:],
                op0=mybir.AluOpType.mult, op1=mybir.AluOpType.add)
            nc.sync.dma_start(out=out_ext[tt * P:(tt + 1) * P, :], in_=o2[:])

    nc.compile()
    return nc


def _split(a):
    h = a.astype(bf)
    l = (a - h.astype(np.float32)).astype(bf)
    return h, l


def _prep_inputs(student_hidden, expert_outputs, params):
    """host-side prep: shard + transpose + hi/lo split + weight layout"""
    p = {k: np.asarray(v, dtype=np.float32) for k, v in params.items()}
    sh = np.ascontiguousarray(np.asarray(student_hidden, np.float32).reshape(NTOK, S_DIM))
    eo = np.asarray(expert_outputs, np.float32).reshape(E, NTOK, T_DIM)

    wmap = {
        "t2s": ("t2s_w", "t2s_b"), "es1": ("es_w1", "es_b1"), "es2": ("es_w2", "es_b2"),
        "cap1": ("cap_w1", "cap_b1"), "cap2": ("cap_w2", "cap_b2"), "cap3": ("cap_w3", "cap_b3"),
        "gap1": ("gap_w1", "gap_b1"), "gap2": ("gap_w2", "gap_b2"),
    }
    shared = {}
    for L, (wk, bk) in wmap.items():
        ind, outd = LAYERS[L]
        w = p[wk]  # [out, in] torch convention
        wT = np.ascontiguousarray(w.T)
        b = p[bk]
        if wT.shape[1] < outd:  # pad logits layers to 128 outputs
            wT = np.concatenate(
                [wT, np.zeros((ind, outd - wT.shape[1]), np.float32)], axis=1)
            b = np.concatenate([b, np.zeros(outd - b.shape[0], np.float32)])
        wh, wl = _split(wT)
        shared[f"{L}_w"] = np.ascontiguousarray(np.stack([wh, wl], axis=1))
        shared[f"{L}_b"] = np.ascontiguousarray(b.reshape(outd, 1))
        if L in LN_LAYERS:
            shared[f"{L}_g"] = np.ascontiguousarray(p[wk.replace("_w", "_g")].reshape(outd, 1))
            ben = {"cap1": "cap_be1", "cap2": "cap_be2", "gap1": "gap_be1"}[L]
            shared[f"{L}_be"] = np.ascontiguousarray(p[ben].reshape(outd, 1))
    shared["iota"] = np.arange(P, dtype=np.float32).reshape(P, 1)
    inv_temp = float(1.0 / p["temp"].reshape(-1)[0])
    shared["cap3_b"] = np.ascontiguousarray(shared["cap3_b"] * inv_temp)

    in_maps = []
    for c in range(NCORES):
        cs = slice(c * CTOK, (c + 1) * CTOK)
        m = dict(shared)
        m["st_h"], m["st_l"] = _split(np.ascontiguousarray(sh[cs].T))
        m["te_h"], m["te_l"] = _split(np.ascontiguousarray(eo[0, cs].T))
        m["eo"] = np.ascontiguousarray(eo[:, cs, :]).reshape(E * CTOK, T_DIM)
        in_maps.append(m)
    return in_maps, inv_temp


_CACHE = {}


def kernel(student_hidden, expert_outputs, params):
    in_maps, inv_temp = _prep_inputs(student_hidden, expert_outputs, params)
    key = ("nc", inv_temp)
    if key not in _CACHE:
        _CACHE[key] = _build(inv_temp)
    nc = _CACHE[key]
    trace = bool(int(os.environ.get("KERNEL_TRACE", "0")))
    tmpdir = os.environ.get("KERNEL_TRACE_DIR") or None
    res = run_bass_kernel_spmd(nc, in_maps, core_ids=list(range(NCORES)),
                               trace=trace, tmpdir=tmpdir)
    if trace:
        kernel.last_exec_time_ns = res.exec_time_ns
        kernel.last_results = res
    out = np.concatenate([res.results[c]["out"] for c in range(NCORES)], axis=0)
    return out.reshape(2, NTOK // 2, T_DIM)


kernel.last_exec_time_ns = None
kernel.last_results = None


# revision 23
# speedup vs baseline: 1.1997x; 1.0259x over previous
"""AdaptiveExpertRouter Trainium2 kernel (8 NeuronCores, data-parallel over tokens).

Per-core pipeline (512 tokens, feature-major activations [feat_part, tok_free]):
  - every Linear is a 3-term bf16 hi/lo split matmul (x@W = xh@Wh + xl@Wh + xh@Wl)
    accumulated in fp32 PSUM -> ~7e-6 relative error, needed so top-2 expert
    selection matches the fp32 reference on realistic score gaps
  - LayerNorm stats via ones-row matmuls (mean from bf16 hi/lo, E[x^2] from an
    fp32r Square); per-token mu/rstd broadcast back through a DRAM round-trip
    DMA (partition-replicating read) so the PE stream never stalls on stats
  - layer emission order interleaves the three scoring branches so independent
    matmul work covers every LN stats barrier and the PE stays at full clock
  - logits transposed to token-major with PE transpose, softmax + top-2 via the
    DVE max8/max_index ops, then an indirect-DMA gather of the 2 selected
    expert rows per token and an fp32 weighted combine.
"""

import os
import sys
from contextlib import ExitStack

for _p in ("/opt/trn_rl_repo", "/root/.axon_site/_ro/trn_rl_repo"):
    if os.path.isdir(_p) and _p not in sys.path:
        sys.path.append(_p)

import numpy as np
import ml_dtypes

import concourse.bass as bass
import concourse.mybir as mybir
import concourse.bacc as bacc
from concourse.tile import TileContext
from concourse.bass_utils import run_bass_kernel_spmd
from concourse.masks import make_identity

P = 128
NCORES = 8
NTOK = 4096            # B*S
CTOK = NTOK // NCORES  # 512 tokens per core
NTT = CTOK // P        # 4 token tiles per core
S_DIM = 1024
T_DIM = 4096
E = 8

f32 = mybir.dt.float32
f32r = mybir.dt.float32r
bf16 = mybir.dt.bfloat16
i32 = mybir.dt.int32
u32 = mybir.dt.uint32

bf = ml_dtypes.bfloat16

# layer name -> (in_dim, out_dim)
LAYERS = {
    "t2s":  (T_DIM, S_DIM),
    "es1":  (T_DIM, 2 * S_DIM),
    "es2":  (2 * S_DIM, P),      # logits padded 8 -> 128
    "cap1": (S_DIM, 2 * S_DIM),
    "cap2": (2 * S_DIM, S_DIM),
    "cap3": (S_DIM, P),          # logits padded
    "gap1": (2 * S_DIM, T_DIM),
    "gap2": (T_DIM, P),          # logits padded
}
LN_LAYERS = {"cap1", "cap2", "gap1"}


def _build(inv_temp: float):
    nc = bacc.Bacc("TRN2", target_bir_lowering=False, debug=False,
                   num_devices=NCORES)

    ext = {}
    ext["st_h"] = nc.dram_tensor("st_h", [S_DIM, CTOK], bf16, kind="ExternalInput")
    ext["st_l"] = nc.dram_tensor("st_l", [S_DIM, CTOK], bf16, kind="ExternalInput")
    ext["te_h"] = nc.dram_tensor("te_h", [T_DIM, CTOK], bf16, kind="ExternalInput")
    ext["te_l"] = nc.dram_tensor("te_l", [T_DIM, CTOK], bf16, kind="ExternalInput")
    for L, (ind, outd) in LAYERS.items():
        ext[f"{L}_w"] = nc.dram_tensor(f"{L}_w", [ind, 2, outd], bf16, kind="ExternalInput")
        ext[f"{L}_b"] = nc.dram_tensor(f"{L}_b", [outd, 1], f32, kind="ExternalInput")
        if L in LN_LAYERS:
            ext[f"{L}_g"] = nc.dram_tensor(f"{L}_g", [outd, 1], f32, kind="ExternalInput")
            ext[f"{L}_be"] = nc.dram_tensor(f"{L}_be", [outd, 1], f32, kind="ExternalInput")
    ext["iota"] = nc.dram_tensor("iota", [P, 1], f32, kind="ExternalInput")
    ext["eo"] = nc.dram_tensor("eo", [E * CTOK, T_DIM], f32, kind="ExternalInput")
    out_ext = nc.dram_tensor("out", [CTOK, T_DIM], f32, kind="ExternalOutput")
    # scratch DRAM for the stats partition-broadcast round trip
    bc_dram = {}
    for L in LN_LAYERS:
        bc_dram[f"{L}_mu"] = nc.dram_tensor(f"{L}_mu_d", [1, CTOK], f32)
        bc_dram[f"{L}_rstd"] = nc.dram_tensor(f"{L}_rstd_d", [1, CTOK], f32)

    with TileContext(nc) as tc, ExitStack() as top:
        const = top.enter_context(tc.tile_pool(name="const", bufs=1))
        ident = const.tile([P, P], f32, name="ident")
        make_identity(nc, ident)
        iota_sb = const.tile([P, 1], f32, name="iota_sb")
        nc.sync.dma_start(out=iota_sb[:], in_=ext["iota"][:])
        eps_t = const.tile([1, 1], f32, name="eps_t")
        nc.vector.memset(eps_t[:], 1e-5)
        ones_bf = {}
        ones_fr = {}
        for D in (S_DIM, 2 * S_DIM, T_DIM):
            tb = const.tile([P, 1], bf16, name=f"ones_bf_{D}")
            nc.vector.memset(tb[:], 1.0 / D)
            ones_bf[D] = tb
            t0 = const.tile([P, 1], f32, name=f"ones_f_{D}")
            nc.vector.memset(t0[:], 1.0 / D)
            tr = const.tile([P, 1], f32r, name=f"ones_fr_{D}")
            nc.vector.tensor_copy(out=tr[:], in_=t0[:])
            ones_fr[D] = tr

        wmix = const.tile([P, 3, 1], f32, name="wmix")
        for bi, wv in enumerate((0.4, 0.3, 0.3)):
            nc.vector.memset(wmix[:, bi, :], wv)
        biasp = top.enter_context(tc.tile_pool(name="biasp", bufs=1))
        wpool = top.enter_context(tc.tile_pool(name="wpool", bufs=6))
        lnt = top.enter_context(tc.tile_pool(name="lnt", bufs=6))
        statp = top.enter_context(tc.tile_pool(name="statp", bufs=2))
        bcp = top.enter_context(tc.tile_pool(name="bcp", bufs=2))
        lgp = top.enter_context(tc.tile_pool(name="lgp", bufs=1))
        tokp = top.enter_context(tc.tile_pool(name="tokp", bufs=40))
        # one shared-tag pool for all bf16 activation tiles: slots recycle
        # dynamically by lifetime, avoiding LIFO pool-stack constraints
        act_ctx = ExitStack()
        actp = act_ctx.enter_context(tc.tile_pool(name="actp", bufs=158))
        psT = top.enter_context(tc.tile_pool(name="psT", bufs=1, space="PSUM"))
        mm_ctx = ExitStack()
        psA = mm_ctx.enter_context(tc.tile_pool(name="psA", bufs=5, space="PSUM"))
        psS = mm_ctx.enter_context(tc.tile_pool(name="psS", bufs=2, space="PSUM"))

        def load_vec(name, outd):
            t = biasp.tile([P, outd // P, 1], f32, name=f"{name}_sb")
            nc.sync.dma_start(
                out=t[:], in_=ext[name][:].rearrange("(ot p) one -> p ot one", p=P))
            return t

        def resident_acts(pool, nm, src_h, src_l, dim):
            """fully-resident activations, one DMA per ktile; returns provider"""
            kt = dim // P
            tiles = []
            for k in range(kt):
                h = pool.tile([P, CTOK], bf16, name=f"{nm}_h{k}", tag="act")
                l = pool.tile([P, CTOK], bf16, name=f"{nm}_l{k}", tag="act")
                nc.sync.dma_start(out=h[:], in_=src_h[k * P:(k + 1) * P, :])
                nc.sync.dma_start(out=l[:], in_=src_l[k * P:(k + 1) * P, :])
                tiles.append((h[:], l[:]))

            def provider(k, og):
                return tiles[k]
            return provider

        def lazy_resident(pool, nm, src_h, src_l):
            """resident tiles DMA'd at first use (streams in under the
            consuming layer's own matmuls, reused by later ogroups)"""
            tiles = {}

            def provider(k, og):
                if k not in tiles:
                    h = pool.tile([P, CTOK], bf16, name=f"{nm}_h{k}", tag="act")
                    l = pool.tile([P, CTOK], bf16, name=f"{nm}_l{k}", tag="act")
                    nc.sync.dma_start(out=h[:], in_=src_h[k * P:(k + 1) * P, :])
                    nc.sync.dma_start(out=l[:], in_=src_l[k * P:(k + 1) * P, :])
                    tiles[k] = (h[:], l[:])
                return tiles[k]
            return provider

        def pair_provider(pairs):
            def provider(k, og):
                return pairs[k]
            return provider

        def split_linear(L, provider, epilogue):
            """3-term split matmuls for layer L; epilogue(ot, psum_ap) per
            128-row output tile. Term order reuses the wh LDWEIGHTS."""
            ind, outd = LAYERS[L]
            nkt = ind // P
            n_ot = outd // P
            for og in range(0, n_ot, 4):
                ots = list(range(og, min(og + 4, n_ot)))
                ps = {}
                for ot in ots:
                    ps[ot] = psA.tile([P, CTOK], f32, name=f"{L}_ps{ot}", tag="psA")
                for kt in range(nkt):
                    cs = slice(ots[0] * P, (ots[-1] + 1) * P)
                    w = wpool.tile([P, 2, len(ots) * P], bf16,
                                   name=f"{L}_w{og}_{kt}", tag="wblk")
                    nc.sync.dma_start(out=w[:], in_=ext[f"{L}_w"][kt * P:(kt + 1) * P, :, cs])
                    xh, xl = provider(kt, og)
                    first = kt == 0
                    last = kt == nkt - 1
                    for j, ot in enumerate(ots):
                        sl = slice(j * P, (j + 1) * P)
                        nc.tensor.matmul(ps[ot][:], lhsT=w[:, 0, sl], rhs=xh,
                                         start=first, stop=False)
                        nc.tensor.matmul(ps[ot][:], lhsT=w[:, 0, sl], rhs=xl,
                                         start=False, stop=False)
                        nc.tensor.matmul(ps[ot][:], lhsT=w[:, 1, sl], rhs=xh,
                                         start=False, stop=last)
                for ot in ots:
                    epilogue(ot, ps[ot][:])

        def plain_split_layer(L, provider, pool, gelu):
            """Linear (+bias) [+gelu], output split to bf16 hi/lo pairs."""
            _, outd = LAYERS[L]
            n_ot = outd // P
            b = load_vec(f"{L}_b", outd)
            hs, ls = [], []

            def epi(ot, psum):
                ba = b[:, ot, :]
                h = pool.tile([P, CTOK], bf16, name=f"{L}_h{ot}", tag="act")
                l = pool.tile([P, CTOK], bf16, name=f"{L}_l{ot}", tag="act")
                if gelu:
                    g32 = lnt.tile([P, CTOK], f32, name=f"{L}_g32_{ot}", tag="lnt")
                    nc.scalar.activation(g32[:], psum,
                                         mybir.ActivationFunctionType.Gelu, bias=ba)
                    nc.scalar.copy(h[:], g32[:])
                    nc.vector.tensor_sub(l[:], g32[:], h[:])
                else:
                    nc.scalar.activation(h[:], psum,
                                         mybir.ActivationFunctionType.Identity, bias=ba)
                    nc.vector.scalar_tensor_tensor(
                        l[:], psum, ba, h[:],
                        op0=mybir.AluOpType.add, op1=mybir.AluOpType.subtract)
                hs.append(h)
                ls.append(l)

            split_linear(L, provider, epi)
            return [(hs[i][:], ls[i][:]) for i in range(n_ot)]

        def ln_layer_mms(L, provider, pool):
            """Emit Linear+bias matmuls and LN stats for layer L; returns a
            finalize() that emits the normalize+GELU+split (call it after
            emitting independent PE work to cover the stats latency)."""
            _, outd = LAYERS[L]
            n_ot = outd // P
            b = load_vec(f"{L}_b", outd)
            g = load_vec(f"{L}_g", outd)
            be = load_vec(f"{L}_be", outd)
            mu_ps = psS.tile([1, CTOK], f32, name=f"{L}_mu", tag="psS")
            m2_ps = psS.tile([1, CTOK], f32, name=f"{L}_m2", tag="psS")
            yhs, yls = [], []

            def epi(ot, psum):
                ba = b[:, ot, :]
                yh = pool.tile([P, CTOK], bf16, name=f"{L}_yh{ot}", tag="act")
                yl = pool.tile([P, CTOK], bf16, name=f"{L}_yl{ot}", tag="act")
                nc.scalar.activation(yh[:], psum,
                                     mybir.ActivationFunctionType.Identity, bias=ba)
                nc.vector.scalar_tensor_tensor(
                    yl[:], psum, ba, yh[:],
                    op0=mybir.AluOpType.add, op1=mybir.AluOpType.subtract)
                sq = lnt.tile([P, CTOK], f32r, name=f"{L}_sq_{ot}", tag="lnt")
                nc.scalar.activation(sq[:], psum,
                                     mybir.ActivationFunctionType.Square, bias=ba)
                first = ot == 0
                last = ot == n_ot - 1
                nc.tensor.matmul(mu_ps[:], lhsT=ones_bf[outd][:], rhs=yh[:],
                                 start=first, stop=False)
                nc.tensor.matmul(mu_ps[:], lhsT=ones_bf[outd][:], rhs=yl[:],
                                 start=False, stop=last)
                nc.tensor.matmul(m2_ps[:], lhsT=ones_fr[outd][:], rhs=sq[:],
                                 start=first, stop=last)
                yhs.append(yh)
                yls.append(yl)

            split_linear(L, provider, epi)

            # stats chain (no PE instructions -> PE stream never stalls here)
            mu = statp.tile([1, CTOK], f32, name=f"{L}_mu_sb", tag="stat")
            nc.vector.tensor_copy(out=mu[:], in_=mu_ps[:])
            var = statp.tile([1, CTOK], f32, name=f"{L}_var", tag="stat")
            nc.vector.tensor_mul(var[:], mu[:], mu[:])
            nc.vector.tensor_sub(var[:], m2_ps[:], var[:])
            std = statp.tile([1, CTOK], f32, name=f"{L}_std", tag="stat")
            nc.scalar.activation(std[:], var[:],
                                 mybir.ActivationFunctionType.Sqrt, bias=eps_t[:])
            rstd = statp.tile([1, CTOK], f32, name=f"{L}_rstd", tag="stat")
            nc.vector.reciprocal(rstd[:], std[:])
            # partition-broadcast via DRAM round trip
            mu_b = bcp.tile([P, CTOK], f32, name=f"{L}_mu_b", tag="bcast")
            rstd_b = bcp.tile([P, CTOK], f32, name=f"{L}_rstd_b", tag="bcast")
            for src, key, dst in ((mu, f"{L}_mu", mu_b), (rstd, f"{L}_rstd", rstd_b)):
                dr = bc_dram[key]
                nc.sync.dma_start(out=dr[:], in_=src[:])
                nc.sync.dma_start(out=dst[:], in_=dr[:].to_broadcast([P, CTOK]))

            def finalize():
                hs, ls = [], []
                for ot in range(n_ot):
                    yh, yl = yhs[ot], yls[ot]
                    t1 = lnt.tile([P, CTOK], f32, name=f"{L}_t1_{ot}", tag="lnt")
                    nc.vector.tensor_sub(t1[:], yh[:], mu_b[:])
                    nc.vector.tensor_add(t1[:], t1[:], yl[:])
                    nc.vector.tensor_mul(t1[:], t1[:], rstd_b[:])
                    g32 = lnt.tile([P, CTOK], f32, name=f"{L}_g32_{ot}", tag="lnt")
                    nc.scalar.activation(g32[:], t1[:],
                                         mybir.ActivationFunctionType.Gelu,
                                         bias=be[:, ot, :], scale=g[:, ot, :])
                    h = pool.tile([P, CTOK], bf16, name=f"{L}_h{ot}", tag="act")
                    l = pool.tile([P, CTOK], bf16, name=f"{L}_l{ot}", tag="act")
                    nc.scalar.copy(h[:], g32[:])
                    nc.vector.tensor_sub(l[:], g32[:], h[:])
                    hs.append(h)
                    ls.append(l)
                return [(hs[i][:], ls[i][:]) for i in range(n_ot)]

            return finalize

        def logits_layer(L, provider, lgp, scale=1.0):
            b = load_vec(f"{L}_b", P)
            res = lgp.tile([P, CTOK], f32, name=f"{L}_lg")

            def epi(ot, psum):
                nc.scalar.activation(res[:], psum,
                                     mybir.ActivationFunctionType.Identity,
                                     bias=b[:, 0, :], scale=scale)

            split_linear(L, provider, epi)
            return res

        # ---------------- layer graph ----------------
        # order chosen so independent matmul streams cover every LN stats
        # barrier: cap1 -> t2s -> [cap1 fin] -> cap2 -> [cap2 fin under gap1]
        # -> gap1 -> cap3 -> es1 (covers gap1 finalize) -> gap2 -> es2
        st = resident_acts(actp, "st", ext["st_h"], ext["st_l"], S_DIM)

        cap1_fin = ln_layer_mms("cap1", st, actp)
        cap1 = cap1_fin()  # no PE ops; DVE work overlaps t2s matmuls below

        te1 = lazy_resident(actp, "te1", ext["te_h"], ext["te_l"])
        t2s = plain_split_layer("t2s", te1, actp, gelu=False)

        cap2_fin = ln_layer_mms("cap2", pair_provider(cap1), actp)
        cap2 = cap2_fin()  # DVE work overlaps gap1 matmuls below

        def gap_in(k, og):
            if k < S_DIM // P:
                return st(k, og)
            return t2s[k - S_DIM // P]

        gap1_fin = ln_layer_mms("gap1", gap_in, actp)

        lt3 = [tokp.tile([P, 3, E], f32, name=f"lt3_{tt}", tag="lt3")
               for tt in range(NTT)]

        def branch_transpose(lg, br):
            for tt in range(NTT):
                pst = psT.tile([P, P], f32, name=f"tr{br}_{tt}", tag="psT")
                nc.tensor.transpose(out=pst[:], in_=lg[:, tt * P:(tt + 1) * P],
                                    identity=ident[:])
                nc.vector.tensor_copy(out=lt3[tt][:, br, :], in_=pst[:, 0:E])

        lg_cap = logits_layer("cap3", pair_provider(cap2), lgp, scale=inv_temp)
        branch_transpose(lg_cap, 0)

        gap1 = gap1_fin()

        te2 = lazy_resident(actp, "te2", ext["te_h"], ext["te_l"])
        es1 = plain_split_layer("es1", te2, actp, gelu=True)

        lg_es = logits_layer("es2", pair_provider(es1), lgp)
        branch_transpose(lg_es, 2)
        lg_gap = logits_layer("gap2", pair_provider(gap1), lgp)
        branch_transpose(lg_gap, 1)

        # ---------------- token-major epilogue ----------------
        act_ctx.close()  # release activation SBUF for gather/out buffers
        mm_ctx.close()  # release matmul-phase PSUM banks
        gp = top.enter_context(tc.tile_pool(name="gp", bufs=4))
        outp = top.enter_context(tc.tile_pool(name="outp", bufs=4))

        for tt in range(NTT):
            # batched softmax over the 3 branches: lt3[tt] is [P, 3, 8]
            lv = lt3[tt][:]
            m3 = tokp.tile([P, 3, 1], f32, name=f"m3_{tt}", tag="tok3")
            nc.vector.reduce_max(out=m3[:], in_=lv, axis=mybir.AxisListType.X)
            d3 = tokp.tile([P, 3, E], f32, name=f"d3_{tt}", tag="lt3")
            nc.vector.tensor_sub(d3[:], lv, m3[:].to_broadcast([P, 3, E]))
            e3 = tokp.tile([P, 3, E], f32, name=f"e3_{tt}", tag="lt3")
            nc.scalar.activation(e3[:], d3[:], mybir.ActivationFunctionType.Exp)
            den3 = tokp.tile([P, 3, 1], f32, name=f"den3_{tt}", tag="tok3")
            nc.vector.reduce_sum(out=den3[:], in_=e3[:], axis=mybir.AxisListType.X)
            rd3 = tokp.tile([P, 3, 1], f32, name=f"rd3_{tt}", tag="tok3")
            nc.vector.reciprocal(rd3[:], den3[:])
            wd3 = tokp.tile([P, 3, 1], f32, name=f"wd3_{tt}", tag="tok3")
            nc.vector.tensor_mul(wd3[:], rd3[:], wmix[:])
            p3 = tokp.tile([P, 3, E], f32, name=f"p3_{tt}", tag="lt3")
            nc.vector.tensor_mul(p3[:], e3[:], wd3[:].to_broadcast([P, 3, E]))
            comb = tokp.tile([P, E], f32, name=f"comb_{tt}", tag="tok")
            nc.vector.tensor_add(comb[:], p3[:, 0, :], p3[:, 1, :])
            nc.vector.tensor_add(comb[:], comb[:], p3[:, 2, :])

            vals = tokp.tile([P, 8], f32, name=f"vals_{tt}", tag="tok")
            nc.vector.max(out=vals[:], in_=comb[:])
            idx = tokp.tile([P, 8], u32, name=f"idx_{tt}", tag="tok")
            nc.vector.max_index(out=idx[:], in_max=vals[:], in_values=comb[:])

            # offsets + gathers first: get the DMAs in flight, then weights
            idf = tokp.tile([P, 2], f32, name=f"idf_{tt}", tag="tok")
            nc.vector.tensor_copy(out=idf[:], in_=idx[:, 0:2])
            off_f = tokp.tile([P, 2], f32, name=f"offf_{tt}", tag="tok")
            nc.vector.tensor_scalar(off_f[:], idf[:], float(CTOK), iota_sb[:],
                                    op0=mybir.AluOpType.mult,
                                    op1=mybir.AluOpType.add)
            nc.vector.tensor_scalar(off_f[:], off_f[:], float(tt * P), None,
                                    op0=mybir.AluOpType.add)
            off_i = tokp.tile([P, 2], i32, name=f"offi_{tt}", tag="tok")
            nc.vector.tensor_copy(out=off_i[:], in_=off_f[:])
            gs = []
            for j in (0, 1):
                gt = gp.tile([P, T_DIM], f32, name=f"g{j}_{tt}", tag="gather")
                nc.gpsimd.indirect_dma_start(
                    out=gt[:], out_offset=None, in_=ext["eo"][:],
                    in_offset=bass.IndirectOffsetOnAxis(ap=off_i[:, j:j + 1], axis=0))
                gs.append(gt)

            den2 = tokp.tile([P, 1], f32, name=f"den2_{tt}", tag="tok1")
            nc.vector.tensor_add(den2[:], vals[:, 0:1], vals[:, 1:2])
            nc.vector.tensor_scalar(den2[:], den2[:], 1e-8, None,
                                    op0=mybir.AluOpType.add)
            rden = tokp.tile([P, 1], f32, name=f"rden_{tt}", tag="tok1")
            nc.vector.reciprocal(rden[:], den2[:])
            w1 = tokp.tile([P, 1], f32, name=f"w1_{tt}", tag="tok1")
            w2 = tokp.tile([P, 1], f32, name=f"w2_{tt}", tag="tok1")
            nc.vector.tensor_mul(w1[:], vals[:, 0:1], rden[:])
            nc.vector.tensor_mul(w2[:], vals[:, 1:2], rden[:])

            o1 = outp.tile([P, T_DIM], f32, name=f"o1_{tt}", tag="outb")
            nc.scalar.activation(o1[:], gs[0][:],
                                 mybir.ActivationFunctionType.Copy,
                                 scale=w1[:])
            o2 = outp.tile([P, T_DIM], f32, name=f"o2_{tt}", tag="outb")
            nc.vector.scalar_tensor_tensor(
                o2[:], gs[1][:], w2[:], o1[:],
                op0=mybir.AluOpType.mult, op1=mybir.AluOpType.add)
            nc.sync.dma_start(out=out_ext[tt * P:(tt + 1) * P, :], in_=o2[:])

    nc.compile()
    return nc


def _split(a):
    h = a.astype(bf)
    l = (a - h.astype(np.float32)).astype(bf)
    return h, l


def _prep_inputs(student_hidden, expert_outputs, params):
    """host-side prep: shard + transpose + hi/lo split + weight layout"""
    p = {k: np.asarray(v, dtype=np.float32) for k, v in params.items()}
    sh = np.ascontiguousarray(np.asarray(student_hidden, np.float32).reshape(NTOK, S_DIM))
    eo = np.asarray(expert_outputs, np.float32).reshape(E, NTOK, T_DIM)

    wmap = {
        "t2s": ("t2s_w", "t2s_b"), "es1": ("es_w1", "es_b1"), "es2": ("es_w2", "es_b2"),
        "cap1": ("cap_w1", "cap_b1"), "cap2": ("cap_w2", "cap_b2"), "cap3": ("cap_w3", "cap_b3"),
        "gap1": ("gap_w1", "gap_b1"), "gap2": ("gap_w2", "gap_b2"),
    }
    shared = {}
    for L, (wk, bk) in wmap.items():
        ind, outd = LAYERS[L]
        w = p[wk]  # [out, in] torch convention
        wT = np.ascontiguousarray(w.T)
        b = p[bk]
        if wT.shape[1] < outd:  # pad logits layers to 128 outputs
            wT = np.concatenate(
                [wT, np.zeros((ind, outd - wT.shape[1]), np.float32)], axis=1)
            b = np.concatenate([b, np.zeros(outd - b.shape[0], np.float32)])
        wh, wl = _split(wT)
        shared[f"{L}_w"] = np.ascontiguousarray(np.stack([wh, wl], axis=1))
        shared[f"{L}_b"] = np.ascontiguousarray(b.reshape(outd, 1))
        if L in LN_LAYERS:
            shared[f"{L}_g"] = np.ascontiguousarray(p[wk.replace("_w", "_g")].reshape(outd, 1))
            ben = {"cap1": "cap_be1", "cap2": "cap_be2", "gap1": "gap_be1"}[L]
            shared[f"{L}_be"] = np.ascontiguousarray(p[ben].reshape(outd, 1))
    shared["iota"] = np.arange(P, dtype=np.float32).reshape(P, 1)
    inv_temp = float(1.0 / p["temp"].reshape(-1)[0])
    shared["cap3_b"] = np.ascontiguousarray(shared["cap3_b"] * inv_temp)

    in_maps = []
    for c in range(NCORES):
        cs = slice(c * CTOK, (c + 1) * CTOK)
        m = dict(shared)
        m["st_h"], m["st_l"] = _split(np.ascontiguousarray(sh[cs].T))
        m["te_h"], m["te_l"] = _split(np.ascontiguousarray(eo[0, cs].T))
        m["eo"] = np.ascontiguousarray(eo[:, cs, :]).reshape(E * CTOK, T_DIM)
        in_maps.append(m)
    return in_maps, inv_temp


_CACHE = {}


def kernel(student_hidden, expert_outputs, params):
    in_maps, inv_temp = _prep_inputs(student_hidden, expert_outputs, params)
    key = ("nc", inv_temp)
    if key not in _CACHE:
        _CACHE[key] = _build(inv_temp)
    nc = _CACHE[key]
    trace = bool(int(os.environ.get("KERNEL_TRACE", "0")))
    tmpdir = os.environ.get("KERNEL_TRACE_DIR") or None
    res = run_bass_kernel_spmd(nc, in_maps, core_ids=list(range(NCORES)),
                               trace=trace, tmpdir=tmpdir)
    if trace:
        kernel.last_exec_time_ns = res.exec_time_ns
        kernel.last_results = res
    out = np.concatenate([res.results[c]["out"] for c in range(NCORES)], axis=0)
    return out.reshape(2, NTOK // 2, T_DIM)


kernel.last_exec_time_ns = None
kernel.last_results = None
